# revision 4
# baseline (speedup 1.0000x reference)
"""Trainium2 Bass kernel for BertSelfAttentionSubstitute (relu^2 attention).

Full (unsharded) inputs in, full output out. Internally shards across 8
NeuronCores: data-parallel over batch (B=4) x tensor-parallel over heads
(16 heads -> 2 groups of 8). Core i handles batch b=i//2, heads
8*(i%2)..8*(i%2)+7.

Per-core device program (all shapes hardcoded):
  inputs:  xt  [1024, 2048]  = hidden[b].T                       (fp32)
           wqt [1024, 512]   = (Wq[rows]/8 ).T  (scale folded)   (fp32)
           wkt [1024, 512]   = Wk[rows].T                        (fp32)
           wvt [1024, 512]   = Wv[rows].T                        (fp32)
  output:  out [512, 2048]   row h*64+d = ctx^T[d, q] for local head h

  Stage B: QT = wqt.T @ xt, KT = wkt.T @ xt  ([512,2048], d_out major)
           V  = xt.T @ wvt                   ([2048,512], token major, bf16)
  Stage C: per (head, k-tile): scoresT = KT_h[:,kslice].T @ QT_h
           probsT = relu(scoresT)^2 (bf16), ctxT += V_h[kslice].T @ probsT
"""

import sys
import numpy as np

sys.path.insert(0, "/opt/trn_rl_repo")

N_CORES = 8
B, S, D_MODEL = 4, 2048, 1024
NH_LOCAL, HD, DOUT = 8, 64, 512  # per-core heads, head dim, d_out slice
P = 128
DIN_CHUNKS = D_MODEL // P  # 8
DOUT_TILES = DOUT // P  # 4
TOKC = 512  # token chunk for projections
NTOKC = S // TOKC  # 4
NK = S // P  # 16 k-tiles
QHALF = 1024  # scores big-tile q width (2 PSUM banks)

_CACHE = {}


def _build():
    if "nc" in _CACHE:
        return _CACHE["nc"]
    import concourse.bass as bass
    import concourse.tile as tile
    from concourse import bacc, mybir

    f32 = mybir.dt.float32
    f32r = mybir.dt.float32r
    bf16 = mybir.dt.bfloat16

    nc = bacc.Bacc("TRN2", target_bir_lowering=False, debug=False,
                   num_devices=N_CORES)
    xt = nc.dram_tensor("xt", [D_MODEL, S], f32r, kind="ExternalInput").ap()
    wqt = nc.dram_tensor("wqt", [D_MODEL, DOUT], f32r, kind="ExternalInput").ap()
    wkt = nc.dram_tensor("wkt", [D_MODEL, DOUT], f32r, kind="ExternalInput").ap()
    wvt = nc.dram_tensor("wvt", [D_MODEL, DOUT], f32r, kind="ExternalInput").ap()
    out = nc.dram_tensor("out", [DOUT, S], f32, kind="ExternalOutput").ap()

    def r(ap):
        return ap

    with tile.TileContext(nc) as tc:
        with tc.tile_pool(name="persist", bufs=1) as persist, \
             tc.tile_pool(name="xtp", bufs=2) as xtp, \
             tc.tile_pool(name="elem", bufs=3) as elem:

            # --- load weights (persistent) ---
            w_tiles = {}
            for wname, wap in (("q", wqt), ("k", wkt), ("v", wvt)):
                for d in range(DIN_CHUNKS):
                    t = persist.tile([P, DOUT], f32r, tag=f"w{wname}{d}", name=f"w{wname}{d}")
                    nc.sync.dma_start(t[:], wap[d * P:(d + 1) * P, :])
                    w_tiles[(wname, d)] = t

            # persistent QT/KT (fp32) and V (bf16)
            qt_sb = [persist.tile([P, S], f32r, tag=f"qt{t}", name=f"qt{t}")
                     for t in range(DOUT_TILES)]
            kt_sb = [persist.tile([P, S], f32r, tag=f"kt{t}", name=f"kt{t}")
                     for t in range(DOUT_TILES)]
            v_sb = [persist.tile([P, DOUT], bf16, tag=f"v{t}", name=f"v{t}")
                    for t in range(S // P)]

            # --- Stage B: projections ---
            with tc.tile_pool(name="psA", bufs=2, space="PSUM") as psA:
                for c in range(NTOKC):
                    xtc = []
                    for d in range(DIN_CHUNKS):
                        t = xtp.tile([P, TOKC], f32r, tag=f"xt{d}", name=f"xt{d}")
                        nc.sync.dma_start(
                            t[:], xt[d * P:(d + 1) * P, c * TOKC:(c + 1) * TOKC])
                        xtc.append(t)
                    # QT / KT: out [dout_tile 128, tok 512]
                    for wname, dst in (("q", qt_sb), ("k", kt_sb)):
                        for tt in range(DOUT_TILES):
                            ps = psA.tile([P, TOKC], f32, tag="proj")
                            for d in range(DIN_CHUNKS):
                                nc.tensor.matmul(
                                    ps[:],
                                    lhsT=r(w_tiles[(wname, d)][:, tt * P:(tt + 1) * P]),
                                    rhs=r(xtc[d][:]),
                                    start=(d == 0), stop=(d == DIN_CHUNKS - 1))
                            nc.scalar.copy(
                                dst[tt][:, c * TOKC:(c + 1) * TOKC], ps[:])
                    # V: out [tok_tile 128, dout 512] -> bf16
                    for tt in range(TOKC // P):
                        ps = psA.tile([P, DOUT], f32, tag="projv")
                        for d in range(DIN_CHUNKS):
                            nc.tensor.matmul(
                                ps[:],
                                lhsT=r(xtc[d][:, tt * P:(tt + 1) * P]),
                                rhs=r(w_tiles[("v", d)][:]),
                                start=(d == 0), stop=(d == DIN_CHUNKS - 1))
                        nc.vector.tensor_copy(v_sb[c * (TOKC // P) + tt][:], ps[:])

            # --- Stage C: attention ---
            with tc.tile_pool(name="psS", bufs=2, space="PSUM") as psS, \
                 tc.tile_pool(name="psC", bufs=1, space="PSUM") as psC:
                for h in range(NH_LOCAL):
                    po = (h % 2) * HD
                    qt_h = qt_sb[h // 2][po:po + HD, :]
                    kt_h = kt_sb[h // 2][po:po + HD, :]
                    ctx = [psC.tile([HD, TOKC], f32, tag=f"ctx{c}", name=f"ctx{c}")
                           for c in range(NTOKC)]
                    for j in range(NK):
                        for half in range(S // QHALF):
                            ps = psS.tile([P, QHALF], f32, tag="s")
                            for cc in range(QHALF // TOKC):
                                q0 = half * QHALF + cc * TOKC
                                nc.tensor.matmul(
                                    ps[:, cc * TOKC:(cc + 1) * TOKC],
                                    lhsT=r(kt_h[:, j * P:(j + 1) * P]),
                                    rhs=r(qt_h[:, q0:q0 + TOKC]),
                                    start=True, stop=True)
                            relu_t = elem.tile([P, QHALF], bf16, tag="relu")
                            nc.scalar.activation(
                                relu_t[:], ps[:],
                                mybir.ActivationFunctionType.Relu)
                            prob_t = elem.tile([P, QHALF], bf16, tag="prob")
                            nc.vector.tensor_mul(prob_t[:], relu_t[:], relu_t[:])
                            for cc in range(QHALF // TOKC):
                                c = half * (QHALF // TOKC) + cc
                                nc.tensor.matmul(
                                    ctx[c][:],
                                    lhsT=v_sb[j][:, h * HD:(h + 1) * HD],
                                    rhs=prob_t[:, cc * TOKC:(cc + 1) * TOKC],
                                    start=(j == 0), stop=(j == NK - 1))
                    ostage = elem.tile([HD, S], f32, tag="ostage", bufs=2,
                                       name="ostage")
                    for c in range(NTOKC):
                        nc.scalar.copy(
                            ostage[:, c * TOKC:(c + 1) * TOKC], ctx[c][:])
                    nc.sync.dma_start(out[h * HD:(h + 1) * HD, :], ostage[:])

    nc.compile()
    _CACHE["nc"] = nc
    return nc


def _in_maps(hidden_states, Wq, Wk, Wv):
    maps = []
    for i in range(N_CORES):
        b = i // 2
        rows = slice(DOUT * (i % 2), DOUT * (i % 2) + DOUT)
        maps.append({
            "xt": np.ascontiguousarray(hidden_states[b].T),
            "wqt": np.ascontiguousarray(Wq[rows].T) / 8.0,
            "wkt": np.ascontiguousarray(Wk[rows].T),
            "wvt": np.ascontiguousarray(Wv[rows].T),
        })
    return maps


def kernel(hidden_states, attention_mask, Wq, bq, Wk, bk, Wv, bv):
    # attention_mask / biases are structurally zero for this problem spec.
    from concourse.bass_utils import run_bass_kernel_spmd

    nc = _build()
    hidden_states = np.asarray(hidden_states, dtype=np.float32)
    maps = _in_maps(hidden_states,
                    np.asarray(Wq, np.float32),
                    np.asarray(Wk, np.float32),
                    np.asarray(Wv, np.float32))
    res = run_bass_kernel_spmd(nc, maps, core_ids=list(range(N_CORES)))
    out = np.empty((B, S, D_MODEL), np.float32)
    for i in range(N_CORES):
        b = i // 2
        cols = slice(DOUT * (i % 2), DOUT * (i % 2) + DOUT)
        out[b, :, cols] = res.results[i]["out"].T
    return out


# revision 18
# speedup vs baseline: 6497.3374x; 6497.3374x over previous
"""Trainium2 Bass kernel for BertSelfAttentionSubstitute (relu^2 attention).

Full (unsharded) inputs in, full output out. Internally shards across 8
NeuronCores: data-parallel over batch (B=4) x tensor-parallel over heads
(16 heads -> 2 groups of 8). Core i handles batch b=i//2, heads
8*(i%2)..8*(i%2)+7.

Per-core device program (all shapes hardcoded):
  inputs:  xt  [1024, 2048]  = hidden[b].T                       (fp32)
           wqt [1024, 512]   = (Wq[rows]/8 ).T  (scale folded)   (fp32)
           wkt [1024, 512]   = Wk[rows].T                        (fp32)
           wvt [1024, 512]   = Wv[rows].T                        (fp32)
  output:  out [512, 2048]   row h*64+d = ctx^T[d, q] for local head h

  Stage B: QT = wqt.T @ xt, KT = wkt.T @ xt  ([512,2048], d_out major)
           V  = xt.T @ wvt                   ([2048,512], token major, bf16)
  Stage C: per (head, k-tile): scoresT = KT_h[:,kslice].T @ QT_h
           probsT = relu(scoresT)^2 (bf16), ctxT += V_h[kslice].T @ probsT

relu^2: ACT does Relu (PSUM fp32 -> SBUF bf16), DVE squares in bf16.
Scores/ctx matmuls run in bf16 (cheap PE weight loads); projections in
f32r. Input DMAs are split across the SP and ACT HWDGE queues.
"""

import sys
import numpy as np

sys.path.insert(0, "/opt/trn_rl_repo")

N_CORES = 8
B, S, D_MODEL = 4, 2048, 1024
NH_LOCAL, HD, DOUT = 8, 64, 512  # per-core heads, head dim, d_out slice
P = 128
DIN_CHUNKS = D_MODEL // P  # 8
DOUT_TILES = DOUT // P  # 4
TOKC = 512  # token chunk for projections
NTOKC = S // TOKC  # 4
NK = S // P  # 16 k-tiles
QHALF = 1024  # scores big-tile q width (2 PSUM banks)

# of every RELU_SPLIT scores tiles, the first RELU_ACT go ACT+DVE-square,
# the rest go fused-DVE
RELU_SPLIT, RELU_ACT = 14, 14  # all relus on ACT
SQ_SPLIT, SQ_POOL = 5, 0       # all squares on DVE
EL_BUFS = 4                    # relu/prob staging depth
COPIES_ACT = False             # QT/KT/ostage copies on DVE

_CACHE = {}


def _emit(nc, tc, mybir, xt, xtb, wqt, wkt, wvt, out, loop_n=None,
          sink=None, seed=None):
    f32 = mybir.dt.float32
    f32r = mybir.dt.float32r
    bf16 = mybir.dt.bfloat16

    with tc.tile_pool(name="persist", bufs=1) as persist, \
         tc.tile_pool(name="xtp", bufs=2) as xtp, \
         tc.tile_pool(name="elem", bufs=EL_BUFS) as elem:

        ones = persist.tile([P, 1], f32, tag="ones", name="ones")
        nc.vector.memset(ones[:], 1.0)

        if seed is not None:
            # timing mode: fill internal DRAM inputs from the small seed
            sx = persist.tile([P, TOKC], f32, tag="seedx", name="seedx")
            sw = persist.tile([P, TOKC], f32, tag="seedw", name="seedw")
            nc.sync.dma_start(sx[:], seed[:, 0:TOKC])
            nc.sync.dma_start(sw[:], seed[:, TOKC:2 * TOKC])
            sxb = persist.tile([P, TOKC], bf16, tag="seedxb", name="seedxb")
            nc.vector.tensor_copy(sxb[:], sx[:])
            swb = persist.tile([P, TOKC], bf16, tag="seedwb", name="seedwb")
            nc.vector.tensor_copy(swb[:], sw[:])
            for d in range(DIN_CHUNKS):
                for c in range(NTOKC):
                    nc.sync.dma_start(
                        xt[d * P:(d + 1) * P, c * TOKC:(c + 1) * TOKC],
                        sx.bitcast(f32r)[:])
                    nc.sync.dma_start(
                        xtb[d * P:(d + 1) * P, c * TOKC:(c + 1) * TOKC],
                        sxb[:])
                for wap in (wqt, wkt):
                    nc.sync.dma_start(wap[d * P:(d + 1) * P, :],
                                      sw.bitcast(f32r)[:])
                nc.sync.dma_start(wvt[d * P:(d + 1) * P, :], swb[:])

        def body():
            # --- load weights ---
            w_tiles = {}
            for wname, wap, wdt in (("q", wqt, f32r), ("k", wkt, f32r),
                                    ("v", wvt, bf16)):
                for d in range(DIN_CHUNKS):
                    t = persist.tile([P, DOUT], wdt, tag=f"w{wname}{d}",
                                     name=f"w{wname}{d}")
                    nc.scalar.dma_start(t[:], wap[d * P:(d + 1) * P, :])
                    w_tiles[(wname, d)] = t

            qt_sb = [persist.tile([P, S], bf16, tag=f"qt{t}", name=f"qt{t}")
                     for t in range(DOUT_TILES)]
            kt_sb = [persist.tile([P, S], bf16, tag=f"kt{t}", name=f"kt{t}")
                     for t in range(DOUT_TILES)]
            v_sb = [persist.tile([P, DOUT], bf16, tag=f"v{t}", name=f"v{t}")
                    for t in range(S // P)]

            # --- Stage B: projections ---
            with tc.tile_pool(name="psA", bufs=2, space="PSUM") as psA:
                for c in range(NTOKC):
                    xtc = []
                    xbc = []
                    for d in range(DIN_CHUNKS):
                        t = xtp.tile([P, TOKC], f32r, tag=f"xt{d}",
                                     name=f"xt{d}")
                        nc.sync.dma_start(
                            t[:], xt[d * P:(d + 1) * P, c * TOKC:(c + 1) * TOKC])
                        xtc.append(t)
                        tb = xtp.tile([P, TOKC], bf16, tag=f"xb{d}",
                                      name=f"xb{d}")
                        nc.sync.dma_start(
                            tb[:], xtb[d * P:(d + 1) * P, c * TOKC:(c + 1) * TOKC])
                        xbc.append(tb)
                    for wname, dst in (("q", qt_sb), ("k", kt_sb)):
                        for tt in range(DOUT_TILES):
                            ps = psA.tile([P, TOKC], f32, tag="proj", name="ps")
                            for d in range(DIN_CHUNKS):
                                nc.tensor.matmul(
                                    ps[:],
                                    lhsT=w_tiles[(wname, d)][:, tt * P:(tt + 1) * P],
                                    rhs=xtc[d][:],
                                    start=(d == 0), stop=(d == DIN_CHUNKS - 1))
                            if COPIES_ACT:
                                nc.scalar.copy(
                                    dst[tt][:, c * TOKC:(c + 1) * TOKC], ps[:])
                            else:
                                nc.vector.tensor_copy(
                                    dst[tt][:, c * TOKC:(c + 1) * TOKC], ps[:])
                    for tt in range(TOKC // P):
                        ps = psA.tile([P, DOUT], f32, tag="projv", name="psv")
                        for d in range(DIN_CHUNKS):
                            nc.tensor.matmul(
                                ps[:],
                                lhsT=xbc[d][:, tt * P:(tt + 1) * P],
                                rhs=w_tiles[("v", d)][:],
                                start=(d == 0), stop=(d == DIN_CHUNKS - 1))
                        nc.vector.tensor_copy(v_sb[c * (TOKC // P) + tt][:], ps[:])

            # --- Stage C: attention ---
            with tc.tile_pool(name="psS", bufs=2, space="PSUM") as psS, \
                 tc.tile_pool(name="psC", bufs=1, space="PSUM") as psC:
                m = 0
                for h in range(NH_LOCAL):
                    po = (h % 2) * HD
                    qt_h = qt_sb[h // 2][po:po + HD, :]
                    kt_h = kt_sb[h // 2][po:po + HD, :]
                    ctx = [psC.tile([HD, TOKC], f32, tag=f"ctx{c}", name=f"ctx{c}")
                           for c in range(NTOKC)]
                    for j in range(NK):
                        for half in range(S // QHALF):
                            ps = psS.tile([P, QHALF], f32, tag="s")
                            for cc in range(QHALF // TOKC):
                                q0 = half * QHALF + cc * TOKC
                                nc.tensor.matmul(
                                    ps[:, cc * TOKC:(cc + 1) * TOKC],
                                    lhsT=kt_h[:, j * P:(j + 1) * P],
                                    rhs=qt_h[:, q0:q0 + TOKC],
                                    start=True, stop=True)
                            prob_t = elem.tile([P, QHALF], bf16, tag="prob")
                            relu_t = elem.tile([P, QHALF], bf16, tag="relu")
                            if m % RELU_SPLIT < RELU_ACT:
                                nc.scalar.activation(
                                    relu_t[:], ps[:],
                                    mybir.ActivationFunctionType.Relu)
                            else:
                                nc.vector.tensor_scalar_max(
                                    relu_t[:], ps[:], 0.0)
                            if m % SQ_SPLIT < SQ_POOL:
                                nc.gpsimd.tensor_mul(
                                    prob_t[:], relu_t[:], relu_t[:])
                            else:
                                nc.vector.tensor_mul(
                                    prob_t[:], relu_t[:], relu_t[:])
                            m += 1
                            for cc in range(QHALF // TOKC):
                                c = half * (QHALF // TOKC) + cc
                                nc.tensor.matmul(
                                    ctx[c][:],
                                    lhsT=v_sb[j][:, h * HD:(h + 1) * HD],
                                    rhs=prob_t[:, cc * TOKC:(cc + 1) * TOKC],
                                    start=(j == 0), stop=(j == NK - 1))
                    ostage = elem.tile([HD, S], f32, tag="ostage", bufs=2,
                                       name="ostage")
                    for c in range(NTOKC):
                        if COPIES_ACT:
                            nc.scalar.copy(
                                ostage[:, c * TOKC:(c + 1) * TOKC], ctx[c][:])
                        else:
                            nc.vector.tensor_copy(
                                ostage[:, c * TOKC:(c + 1) * TOKC], ctx[c][:])
                    nc.scalar.dma_start(out[h * HD:(h + 1) * HD, :], ostage[:])

        if loop_n is not None:
            with tc.For_i(0, loop_n, 1):
                body()
        else:
            body()



def _build(loop_n=None, internal_io=False):
    key = ("nc", loop_n, internal_io)
    if key in _CACHE:
        return _CACHE[key]
    import concourse.tile as tile
    from concourse import bacc, mybir

    f32 = mybir.dt.float32
    f32r = mybir.dt.float32r

    nc = bacc.Bacc("TRN2", target_bir_lowering=False, debug=False,
                   num_devices=N_CORES)
    ikind = "Internal" if internal_io else "ExternalInput"
    okind = "ExternalOutput"
    bf16 = mybir.dt.bfloat16
    xt = nc.dram_tensor("xt", [D_MODEL, S], f32r, kind=ikind).ap()
    xtb = nc.dram_tensor("xtb", [D_MODEL, S], bf16, kind=ikind).ap()
    wqt = nc.dram_tensor("wqt", [D_MODEL, DOUT], f32r, kind=ikind).ap()
    wkt = nc.dram_tensor("wkt", [D_MODEL, DOUT], f32r, kind=ikind).ap()
    wvt = nc.dram_tensor("wvt", [D_MODEL, DOUT], bf16, kind=ikind).ap()
    out = nc.dram_tensor("out", [DOUT, S], f32, kind=okind).ap()
    sink = None
    seed = None
    if internal_io:
        seed = nc.dram_tensor("seed", [P, 2 * TOKC], f32,
                              kind="ExternalInput").ap()

    with tile.TileContext(nc) as tc:
        _emit(nc, tc, mybir, xt, xtb, wqt, wkt, wvt, out, loop_n=loop_n,
              sink=sink, seed=seed)

    nc.compile()
    _CACHE[key] = nc
    return nc


def _in_maps(hidden_states, Wq, Wk, Wv):
    import ml_dtypes
    maps = []
    for i in range(N_CORES):
        b = i // 2
        rows = slice(DOUT * (i % 2), DOUT * (i % 2) + DOUT)
        xt = np.ascontiguousarray(hidden_states[b].T)
        maps.append({
            "xt": xt,
            "xtb": xt.astype(ml_dtypes.bfloat16),
            "wqt": np.ascontiguousarray(Wq[rows].T) / 8.0,
            "wkt": np.ascontiguousarray(Wk[rows].T),
            "wvt": np.ascontiguousarray(Wv[rows].T).astype(ml_dtypes.bfloat16),
        })
    return maps


def kernel(hidden_states, attention_mask, Wq, bq, Wk, bk, Wv, bv):
    # attention_mask / biases are structurally zero for this problem spec.
    from concourse.bass_utils import run_bass_kernel_spmd

    nc = _build()
    hidden_states = np.asarray(hidden_states, dtype=np.float32)
    maps = _in_maps(hidden_states,
                    np.asarray(Wq, np.float32),
                    np.asarray(Wk, np.float32),
                    np.asarray(Wv, np.float32))
    res = run_bass_kernel_spmd(nc, maps, core_ids=list(range(N_CORES)))
    out = np.empty((B, S, D_MODEL), np.float32)
    for i in range(N_CORES):
        b = i // 2
        cols = slice(DOUT * (i % 2), DOUT * (i % 2) + DOUT)
        out[b, :, cols] = res.results[i]["out"].T
    return out
